# revision 46
# baseline (speedup 1.0000x reference)
"""MoE layer (B=4,S=2048,D=1024,H=4096,E=8,K=2) on 8 trn2 NeuronCores.

Sharding strategy (hardcoded): expert-parallel with a fixed per-expert
capacity of 2048 tokens (= the perfectly load-balanced share: 8192 tokens
x top-2 / 8 experts). Host computes the gate (logits -> top-2 -> softmax
weights) and dispatches: core e receives up to 2048 tokens routed to
expert e (gathered + transposed + padded to the static capacity), plus
expert e's FFN weights in bf16. Each core runs the expert FFN
(x @ W1 -> gelu -> @ W2, fp32 PSUM accumulation) and scales rows by the
combine weight on-device. Tokens beyond an expert's capacity (the
capacity-factor-1.0 overflow, ~2% of pairs) take the host overflow path
(exact fp32 FFN). Host scatter-adds the weighted per-expert outputs back
into the full [B,S,D] output (the "all-to-all combine"), adding the b2
contribution exactly once per (token, expert) pair.

All device inputs are host-packed into SBUF layout ([128 partitions,
contiguous free dim]) so every DMA is 128 descriptors of >=1KB contiguous
runs: descriptor-generation (DIRECT2D) drops from ~5.5us to ~0.7us per
transfer and the transfers run at line rate.
"""

import sys

for _p in ("/opt/trn_rl_repo", "/root/.axon_site"):
    if _p not in sys.path:
        sys.path.insert(0, _p)

import numpy as np
import ml_dtypes

import concourse.bacc as bacc
import concourse.mybir as mybir
import concourse.tile as tile
from concourse.bass_utils import run_bass_kernel_spmd

BF16 = mybir.dt.bfloat16
F32 = mybir.dt.float32

N_CORES = 8
D = 1024
H = 4096
E = 8

_CACHE: dict = {}
LAST_RESULTS = None  # BassKernelResults of the most recent run (for test.py)
TRACE = False  # test.py can flip this to get an NTFF profile

try:
    from scipy.special import erf as _erf
except ImportError:
    import math

    def _erf(a):
        return np.vectorize(math.erf, otypes=[np.float32])(a)


def _build(capT, with_b1):
    nc = bacc.Bacc("TRN2", target_bir_lowering=False, debug=False,
                   num_devices=N_CORES)

    nblk = capT // 512
    # Packed layouts (built host-side):
    #   xp[p, blk, k, c]    = x_flat[token blk*512+c, d=k*128+p]
    #   w1p[p, g, q, k, c]  = W1[d=k*128+p, h=g*512+q*128+c]
    #   w2p[p, g, j, c]     = W2[h=(g*8+j)*128+p, d=c]
    # w1 is quarter(128-col)-granular so the first-matmul gate is only
    # x block0 (1MB) + one quarter (256KB); whole-group DMAs stay one
    # contiguous 8KB-run-per-partition transfer.
    xp_d = nc.dram_tensor("xp", [128, nblk, 8, 512], BF16, kind="ExternalInput")
    w1p_d = nc.dram_tensor("w1p", [128, 8, 4, 8, 128], BF16,
                           kind="ExternalInput")
    w2p_d = nc.dram_tensor("w2p", [128, 4, 8, 1024], BF16, kind="ExternalInput")
    wv_d = nc.dram_tensor("wv", [128, capT // 128], F32, kind="ExternalInput")
    if with_b1:
        b1_d = nc.dram_tensor("b1t", [128, 32], F32, kind="ExternalInput")
    # y ships back as bf16: halves the store traffic and the final store's
    # tail latency; the host combine accumulates in fp32 anyway.
    y_d = nc.dram_tensor("y", [capT, D], BF16, kind="ExternalOutput")

    with tile.TileContext(nc) as tc:
        with (
            tc.tile_pool(name="weights", bufs=1) as wpool,
            tc.tile_pool(name="xin", bufs=1) as xpool,
            tc.tile_pool(name="hbuf", bufs=2) as hpool,
            tc.tile_pool(name="yout", bufs=3) as ypool,
            tc.tile_pool(name="small", bufs=1) as spool,
            tc.tile_pool(name="ps1", bufs=3, space="PSUM") as ps1pool,
            tc.tile_pool(name="ps2", bufs=2, space="PSUM") as ps2pool,
        ):
            # Gating for the first L1 group is ~2MB (x block0 + w1 group0);
            # split it across the two HWDGE queues so each carries ~1MB and
            # the first matmuls can trickle on partial (k-sliced) data.
            xsb = {}
            xsb[0] = xpool.tile([128, 8, 512], BF16, tag="xT", name="xT0")
            w1g = []
            for g in range(8):
                t = wpool.tile([128, 4, 8, 128], BF16, tag=f"w1g{g}",
                               name=f"w1g{g}")
                w1g.append(t)
            # Queue balance: sync carries [q0, x k4-7, then w1 groups 1-7];
            # scalar carries [x k0-3, q1, q2q3] so sync reaches w1g1 (the
            # first steady-state weight group) ~1.5us sooner.
            nc.sync.dma_start(w1g[0][:, 0], w1p_d[:, 0, 0])
            nc.scalar.dma_start(xsb[0][:, :4, :], xp_d[:, 0, :4])
            nc.sync.dma_start(xsb[0][:, 4:, :], xp_d[:, 0, 4:])
            nc.scalar.dma_start(w1g[0][:, 1], w1p_d[:, 0, 1])
            nc.scalar.dma_start(w1g[0][:, 2:], w1p_d[:, 0, 2:])
            if with_b1:
                b1_sb = spool.tile([128, 32], F32)
                nc.scalar.dma_start(b1_sb[:], b1_d[:])
            for g in range(1, 8):
                nc.sync.dma_start(w1g[g][:], w1p_d[:, g])
            w2g = []
            for g in range(4):
                t = wpool.tile([128, 8, 1024], BF16, tag=f"w2g{g}", name=f"w2g{g}")
                nc.sync.dma_start(t[:], w2p_d[:, g])
                w2g.append(t)

            wv_sb = spool.tile([128, capT // 128], F32)
            nc.sync.dma_start(wv_sb[:], wv_d[:])

            # HAM warm-up: the PE clock sits at 1.2GHz until it has been
            # busy ~3.4us. Spend that window on dummy matmuls while the
            # gating DMAs are in flight; sized so the burst ends right as
            # the first real tiles land (~12us), handing off without an
            # idle gap (an idle gap >3.4us would re-throttle the clock).
            warm_src = spool.tile([128, 128], BF16, name="warm_src")
            nc.gpsimd.memset(warm_src[:], 0.0)
            warm_ps = ps1pool.tile([128, 2, 512], F32, tag="ps1",
                                   name="warm_ps", bufs=None)
            for wi in range(31):
                nc.tensor.matmul(
                    warm_ps[:64, 0, :128], warm_src[:, :64], warm_src[:],
                    start=True, stop=True, skip_group_check=True)

            for blk in range(nblk):
                t0, tn = blk * 512, 512
                if blk not in xsb:
                    xsb[blk] = xpool.tile([128, 8, 512], BF16, tag="xT",
                                          name=f"xT{blk}")
                    nc.sync.dma_start(xsb[blk][:], xp_d[:, blk])
                xt = xsb[blk]

                # ---- layer 1: hT[m*128:(m+1)*128, :tn] for 32 H-tiles ----
                hT = hpool.tile([128, 32, 512], BF16, tag="hT", name=f"hT{blk}")
                for mg in range(16):
                    ps1 = ps1pool.tile([128, 2, 512], F32, tag="ps1",
                                       name=f"ps1_{blk}_{mg}")
                    for mj in range(2):
                        m = mg * 2 + mj
                        lg, q = m // 4, m % 4
                        for k in range(8):
                            nc.tensor.matmul(
                                ps1[:, mj, :tn],
                                w1g[lg][:, q, k, :],
                                xt[:, k, :tn],
                                start=(k == 0), stop=(k == 7),
                            )
                    if with_b1:
                        for mj in range(2):
                            m = mg * 2 + mj
                            nc.scalar.activation(
                                hT[:, m, :tn], ps1[:, mj, :tn],
                                mybir.ActivationFunctionType.Gelu,
                                bias=b1_sb[:, m:m + 1],
                            )
                    else:
                        nc.scalar.activation(
                            hT[:, mg * 2:mg * 2 + 2, :tn], ps1[:, :, :tn],
                            mybir.ActivationFunctionType.Gelu,
                        )

                # ---- layer 2: y[t0+tm*128 ..., :] = hT.T @ W2, scaled ----
                for tm in range(tn // 128):
                    col = t0 // 128 + tm
                    rows = slice(t0 + tm * 128, t0 + (tm + 1) * 128)
                    for dn in range(2):
                        ps2 = ps2pool.tile([128, 512], F32, tag="ps2",
                                           name=f"ps2_{blk}_{tm}_{dn}")
                        for h in range(32):
                            nc.tensor.matmul(
                                ps2[:, :],
                                hT[:, h, tm * 128:(tm + 1) * 128],
                                w2g[h // 8][:, h % 8, dn * 512:(dn + 1) * 512],
                                start=(h == 0), stop=(h == 31),
                            )
                        yt = ypool.tile([128, 512], BF16, tag="yt",
                                        name=f"yt_{blk}_{tm}_{dn}")
                        if blk == nblk - 1 and tm == tn // 128 - 1 and dn == 1:
                            # Tail: scale on ACT (faster PSUM read) and
                            # store from the same engine's queue, avoiding
                            # the cross-engine handoff on the last chain.
                            nc.scalar.activation(
                                yt[:], ps2[:],
                                mybir.ActivationFunctionType.Copy,
                                scale=wv_sb[:, col:col + 1])
                            nc.scalar.dma_start(y_d[rows, 512:1024], yt[:])
                        else:
                            nc.vector.tensor_scalar_mul(
                                yt[:], ps2[:], wv_sb[:, col:col + 1])
                            nc.sync.dma_start(
                                y_d[rows, dn * 512:(dn + 1) * 512], yt[:])

    nc.compile()
    return nc


def _route(x_flat, Wg, bg):
    """Host gate: returns per-expert (token_idx, combine_weight)."""
    logits = x_flat @ Wg.astype(np.float32) + bg.astype(np.float32)
    T = logits.shape[0]
    ar = np.arange(T)
    top1 = np.argmax(logits, axis=1)
    l2 = logits.copy()
    l2[ar, top1] = -np.inf
    top2 = np.argmax(l2, axis=1)
    v1 = logits[ar, top1]
    v2 = logits[ar, top2]
    # softmax over the two selected logits (v1 >= v2)
    e2 = np.exp(v2 - v1)
    s = 1.0 + e2
    wt1 = (1.0 / s).astype(np.float32)
    wt2 = (e2 / s).astype(np.float32)
    idx, wgt = [], []
    for e in range(E):
        m1 = top1 == e
        m2 = top2 == e
        ii = np.concatenate([ar[m1], ar[m2]])
        ww = np.concatenate([wt1[m1], wt2[m2]])
        order = np.argsort(ii, kind="stable")
        idx.append(ii[order])
        wgt.append(ww[order])
    return idx, wgt


def kernel(x, Wg, bg, W1, b1, W2, b2, _trace=None):
    global LAST_RESULTS
    x = np.asarray(x, dtype=np.float32)
    Wg = np.asarray(Wg, dtype=np.float32)
    bg = np.asarray(bg, dtype=np.float32)
    W1 = np.asarray(W1, dtype=np.float32)
    b1 = np.asarray(b1, dtype=np.float32)
    W2 = np.asarray(W2, dtype=np.float32)
    b2 = np.asarray(b2, dtype=np.float32)

    B, S, _D = x.shape
    T = B * S
    x_flat = np.ascontiguousarray(x.reshape(T, _D))

    idx, wgt = _route(x_flat, Wg, bg)
    counts = [len(i) for i in idx]
    # Fixed capacity = the load-balanced share (capacity factor 1.0).
    # Every core computes capT tokens regardless, so padding all cores to
    # the max expert count just burns PE time on zeros; overflow beyond
    # capT takes the exact host path instead.
    capT = min(2048, max(512, -(-max(counts) // 512) * 512))
    dev_counts = [min(c, capT) for c in counts]
    nblk = capT // 512

    with_b1 = bool(np.any(b1))
    key = (capT, with_b1)
    if key not in _CACHE:
        _CACHE[key] = _build(capT, with_b1)
    nc = _CACHE[key]

    bf = ml_dtypes.bfloat16
    in_maps = []
    for e in range(E):
        cnt = dev_counts[e]
        xT = np.zeros((D, capT), dtype=bf)
        if cnt:
            xT[:, :cnt] = x_flat[idx[e][:cnt]].T
        xp = xT.reshape(8, 128, nblk, 512).transpose(1, 2, 0, 3)
        w1p = W1[e].astype(bf).reshape(8, 128, 8, 4, 128).transpose(
            1, 2, 3, 0, 4)
        w2p = W2[e].astype(bf).reshape(4, 8, 128, 1024).transpose(2, 0, 1, 3)
        wv = np.zeros((capT // 128, 128), dtype=np.float32)
        if cnt:
            wv.reshape(-1)[:cnt] = wgt[e][:cnt]
        m = {
            "xp": np.ascontiguousarray(xp),
            "w1p": np.ascontiguousarray(w1p),
            "w2p": np.ascontiguousarray(w2p),
            "wv": np.ascontiguousarray(wv.T),
        }
        if with_b1:
            m["b1t"] = np.ascontiguousarray(b1[e].reshape(32, 128).T)
        in_maps.append(m)

    do_trace = TRACE if _trace is None else _trace
    res = run_bass_kernel_spmd(nc, in_maps, list(range(N_CORES)),
                               trace=do_trace)
    LAST_RESULTS = res

    out = np.zeros((T, D), dtype=np.float32)
    for e in range(E):
        cnt = dev_counts[e]
        if not cnt:
            continue
        ye = res.results[e]["y"][:cnt].astype(np.float32)
        if np.any(b2[e]):
            ye = ye + np.outer(wgt[e][:cnt], b2[e])
        out[idx[e][:cnt]] += ye

    # Host overflow path: exact fp32 FFN for tokens beyond expert capacity.
    for e in range(E):
        if counts[e] <= capT:
            continue
        oi = idx[e][capT:]
        ow = wgt[e][capT:]
        h = x_flat[oi] @ W1[e] + b1[e]
        h = 0.5 * h * (1.0 + _erf(h * np.float32(0.7071067811865476)))
        ye = h @ W2[e] + b2[e]
        out[oi] += ow[:, None] * ye

    return out.reshape(B, S, D)


# revision 47
# speedup vs baseline: 1.0793x; 1.0793x over previous
"""MoE layer (B=4,S=2048,D=1024,H=4096,E=8,K=2) on 8 trn2 NeuronCores.

Sharding strategy (hardcoded): expert-parallel with a fixed per-expert
capacity of 2048 tokens (= the perfectly load-balanced share: 8192 tokens
x top-2 / 8 experts). Host computes the gate (logits -> top-2 -> softmax
weights) and dispatches: core e receives up to 2048 tokens routed to
expert e (gathered + transposed + padded to the static capacity), plus
expert e's FFN weights in bf16. Each core runs the expert FFN
(x @ W1 -> gelu -> @ W2, fp32 PSUM accumulation) and scales rows by the
combine weight on-device. Tokens beyond an expert's capacity (the
capacity-factor-1.0 overflow, ~2% of pairs) take the host overflow path
(exact fp32 FFN). Host scatter-adds the weighted per-expert outputs back
into the full [B,S,D] output (the "all-to-all combine"), adding the b2
contribution exactly once per (token, expert) pair.

All device inputs are host-packed into SBUF layout ([128 partitions,
contiguous free dim]) so every DMA is 128 descriptors of >=1KB contiguous
runs: descriptor-generation (DIRECT2D) drops from ~5.5us to ~0.7us per
transfer and the transfers run at line rate.
"""

import sys

for _p in ("/opt/trn_rl_repo", "/root/.axon_site"):
    if _p not in sys.path:
        sys.path.insert(0, _p)

import numpy as np
import ml_dtypes

import concourse.bacc as bacc
import concourse.mybir as mybir
import concourse.tile as tile
from concourse.bass_utils import run_bass_kernel_spmd

BF16 = mybir.dt.bfloat16
F32 = mybir.dt.float32

N_CORES = 8
D = 1024
H = 4096
E = 8

_CACHE: dict = {}
LAST_RESULTS = None  # BassKernelResults of the most recent run (for test.py)
TRACE = False  # test.py can flip this to get an NTFF profile

try:
    from scipy.special import erf as _erf
except ImportError:
    import math

    def _erf(a):
        return np.vectorize(math.erf, otypes=[np.float32])(a)


def _build(capT, with_b1):
    nc = bacc.Bacc("TRN2", target_bir_lowering=False, debug=False,
                   num_devices=N_CORES)

    nblk = capT // 512
    # Packed layouts (built host-side):
    #   xp[p, blk, k, c]    = x_flat[token blk*512+c, d=k*128+p]
    #   w1p[p, g, q, k, c]  = W1[d=k*128+p, h=g*512+q*128+c]
    #   w2p[p, g, j, c]     = W2[h=(g*8+j)*128+p, d=c]
    # w1 is quarter(128-col)-granular so the first-matmul gate is only
    # x block0 (1MB) + one quarter (256KB); whole-group DMAs stay one
    # contiguous 8KB-run-per-partition transfer.
    xp_d = nc.dram_tensor("xp", [128, nblk, 8, 512], BF16, kind="ExternalInput")
    w1p_d = nc.dram_tensor("w1p", [128, 8, 4, 8, 128], BF16,
                           kind="ExternalInput")
    w2p_d = nc.dram_tensor("w2p", [128, 4, 8, 1024], BF16, kind="ExternalInput")
    wv_d = nc.dram_tensor("wv", [128, capT // 128], F32, kind="ExternalInput")
    if with_b1:
        b1_d = nc.dram_tensor("b1t", [128, 32], F32, kind="ExternalInput")
    # y ships back as bf16: halves the store traffic and the final store's
    # tail latency; the host combine accumulates in fp32 anyway.
    y_d = nc.dram_tensor("y", [capT, D], BF16, kind="ExternalOutput")

    with tile.TileContext(nc) as tc:
        with (
            tc.tile_pool(name="weights", bufs=1) as wpool,
            tc.tile_pool(name="xin", bufs=1) as xpool,
            tc.tile_pool(name="hbuf", bufs=2) as hpool,
            tc.tile_pool(name="yout", bufs=3) as ypool,
            tc.tile_pool(name="small", bufs=1) as spool,
            tc.tile_pool(name="ps1", bufs=3, space="PSUM") as ps1pool,
            tc.tile_pool(name="ps2", bufs=2, space="PSUM") as ps2pool,
        ):
            # Gating for the first L1 group is ~2MB (x block0 + w1 group0);
            # split it across the two HWDGE queues so each carries ~1MB and
            # the first matmuls can trickle on partial (k-sliced) data.
            xsb = {}
            xsb[0] = xpool.tile([128, 8, 512], BF16, tag="xT", name="xT0")
            w1g = []
            for g in range(8):
                t = wpool.tile([128, 4, 8, 128], BF16, tag=f"w1g{g}",
                               name=f"w1g{g}")
                w1g.append(t)
            nc.sync.dma_start(w1g[0][:, 0], w1p_d[:, 0, 0])
            nc.scalar.dma_start(xsb[0][:, :4, :], xp_d[:, 0, :4])
            nc.sync.dma_start(xsb[0][:, 4:, :], xp_d[:, 0, 4:])
            nc.scalar.dma_start(w1g[0][:, 1], w1p_d[:, 0, 1])
            nc.sync.dma_start(w1g[0][:, 2:], w1p_d[:, 0, 2:])
            if with_b1:
                b1_sb = spool.tile([128, 32], F32)
                nc.scalar.dma_start(b1_sb[:], b1_d[:])
            for g in range(1, 8):
                nc.sync.dma_start(w1g[g][:], w1p_d[:, g])
            w2g = []
            for g in range(4):
                t = wpool.tile([128, 8, 1024], BF16, tag=f"w2g{g}", name=f"w2g{g}")
                nc.sync.dma_start(t[:], w2p_d[:, g])
                w2g.append(t)

            wv_sb = spool.tile([128, capT // 128], F32)
            nc.sync.dma_start(wv_sb[:], wv_d[:])

            # HAM warm-up: the PE clock sits at 1.2GHz until it has been
            # busy ~3.4us. Spend that window on dummy matmuls while the
            # gating DMAs are in flight; sized so the burst ends right as
            # the first real tiles land (~12us), handing off without an
            # idle gap (an idle gap >3.4us would re-throttle the clock).
            warm_src = spool.tile([128, 128], BF16, name="warm_src")
            nc.gpsimd.memset(warm_src[:], 0.0)
            warm_ps = ps1pool.tile([128, 2, 512], F32, tag="ps1",
                                   name="warm_ps", bufs=None)
            for wi in range(34):
                nc.tensor.matmul(
                    warm_ps[:64, 0, :128], warm_src[:, :64], warm_src[:],
                    start=True, stop=True, skip_group_check=True)

            for blk in range(nblk):
                t0, tn = blk * 512, 512
                if blk not in xsb:
                    xsb[blk] = xpool.tile([128, 8, 512], BF16, tag="xT",
                                          name=f"xT{blk}")
                    nc.sync.dma_start(xsb[blk][:], xp_d[:, blk])
                xt = xsb[blk]

                # ---- layer 1: hT[m*128:(m+1)*128, :tn] for 32 H-tiles ----
                hT = hpool.tile([128, 32, 512], BF16, tag="hT", name=f"hT{blk}")
                for mg in range(16):
                    ps1 = ps1pool.tile([128, 2, 512], F32, tag="ps1",
                                       name=f"ps1_{blk}_{mg}")
                    for mj in range(2):
                        m = mg * 2 + mj
                        lg, q = m // 4, m % 4
                        for k in range(8):
                            nc.tensor.matmul(
                                ps1[:, mj, :tn],
                                w1g[lg][:, q, k, :],
                                xt[:, k, :tn],
                                start=(k == 0), stop=(k == 7),
                            )
                    if with_b1:
                        for mj in range(2):
                            m = mg * 2 + mj
                            nc.scalar.activation(
                                hT[:, m, :tn], ps1[:, mj, :tn],
                                mybir.ActivationFunctionType.Gelu,
                                bias=b1_sb[:, m:m + 1],
                            )
                    else:
                        nc.scalar.activation(
                            hT[:, mg * 2:mg * 2 + 2, :tn], ps1[:, :, :tn],
                            mybir.ActivationFunctionType.Gelu,
                        )

                # ---- layer 2: y[t0+tm*128 ..., :] = hT.T @ W2, scaled ----
                for tm in range(tn // 128):
                    col = t0 // 128 + tm
                    rows = slice(t0 + tm * 128, t0 + (tm + 1) * 128)
                    for dn in range(2):
                        ps2 = ps2pool.tile([128, 512], F32, tag="ps2",
                                           name=f"ps2_{blk}_{tm}_{dn}")
                        for h in range(32):
                            nc.tensor.matmul(
                                ps2[:, :],
                                hT[:, h, tm * 128:(tm + 1) * 128],
                                w2g[h // 8][:, h % 8, dn * 512:(dn + 1) * 512],
                                start=(h == 0), stop=(h == 31),
                            )
                        yt = ypool.tile([128, 512], BF16, tag="yt",
                                        name=f"yt_{blk}_{tm}_{dn}")
                        if blk == nblk - 1 and tm == tn // 128 - 1 and dn == 1:
                            # Tail: scale on ACT (faster PSUM read) and
                            # store from the same engine's queue, avoiding
                            # the cross-engine handoff on the last chain.
                            nc.scalar.activation(
                                yt[:], ps2[:],
                                mybir.ActivationFunctionType.Copy,
                                scale=wv_sb[:, col:col + 1])
                            nc.scalar.dma_start(y_d[rows, 512:1024], yt[:])
                        else:
                            nc.vector.tensor_scalar_mul(
                                yt[:], ps2[:], wv_sb[:, col:col + 1])
                            nc.sync.dma_start(
                                y_d[rows, dn * 512:(dn + 1) * 512], yt[:])

    nc.compile()
    return nc


def _route(x_flat, Wg, bg):
    """Host gate: returns per-expert (token_idx, combine_weight)."""
    logits = x_flat @ Wg.astype(np.float32) + bg.astype(np.float32)
    T = logits.shape[0]
    ar = np.arange(T)
    top1 = np.argmax(logits, axis=1)
    l2 = logits.copy()
    l2[ar, top1] = -np.inf
    top2 = np.argmax(l2, axis=1)
    v1 = logits[ar, top1]
    v2 = logits[ar, top2]
    # softmax over the two selected logits (v1 >= v2)
    e2 = np.exp(v2 - v1)
    s = 1.0 + e2
    wt1 = (1.0 / s).astype(np.float32)
    wt2 = (e2 / s).astype(np.float32)
    idx, wgt = [], []
    for e in range(E):
        m1 = top1 == e
        m2 = top2 == e
        ii = np.concatenate([ar[m1], ar[m2]])
        ww = np.concatenate([wt1[m1], wt2[m2]])
        order = np.argsort(ii, kind="stable")
        idx.append(ii[order])
        wgt.append(ww[order])
    return idx, wgt


def kernel(x, Wg, bg, W1, b1, W2, b2, _trace=None):
    global LAST_RESULTS
    x = np.asarray(x, dtype=np.float32)
    Wg = np.asarray(Wg, dtype=np.float32)
    bg = np.asarray(bg, dtype=np.float32)
    W1 = np.asarray(W1, dtype=np.float32)
    b1 = np.asarray(b1, dtype=np.float32)
    W2 = np.asarray(W2, dtype=np.float32)
    b2 = np.asarray(b2, dtype=np.float32)

    B, S, _D = x.shape
    T = B * S
    x_flat = np.ascontiguousarray(x.reshape(T, _D))

    idx, wgt = _route(x_flat, Wg, bg)
    counts = [len(i) for i in idx]
    # Fixed capacity = the load-balanced share (capacity factor 1.0).
    # Every core computes capT tokens regardless, so padding all cores to
    # the max expert count just burns PE time on zeros; overflow beyond
    # capT takes the exact host path instead.
    capT = min(2048, max(512, -(-max(counts) // 512) * 512))
    dev_counts = [min(c, capT) for c in counts]
    nblk = capT // 512

    with_b1 = bool(np.any(b1))
    key = (capT, with_b1)
    if key not in _CACHE:
        _CACHE[key] = _build(capT, with_b1)
    nc = _CACHE[key]

    bf = ml_dtypes.bfloat16
    in_maps = []
    for e in range(E):
        cnt = dev_counts[e]
        xT = np.zeros((D, capT), dtype=bf)
        if cnt:
            xT[:, :cnt] = x_flat[idx[e][:cnt]].T
        xp = xT.reshape(8, 128, nblk, 512).transpose(1, 2, 0, 3)
        w1p = W1[e].astype(bf).reshape(8, 128, 8, 4, 128).transpose(
            1, 2, 3, 0, 4)
        w2p = W2[e].astype(bf).reshape(4, 8, 128, 1024).transpose(2, 0, 1, 3)
        wv = np.zeros((capT // 128, 128), dtype=np.float32)
        if cnt:
            wv.reshape(-1)[:cnt] = wgt[e][:cnt]
        m = {
            "xp": np.ascontiguousarray(xp),
            "w1p": np.ascontiguousarray(w1p),
            "w2p": np.ascontiguousarray(w2p),
            "wv": np.ascontiguousarray(wv.T),
        }
        if with_b1:
            m["b1t"] = np.ascontiguousarray(b1[e].reshape(32, 128).T)
        in_maps.append(m)

    do_trace = TRACE if _trace is None else _trace
    res = run_bass_kernel_spmd(nc, in_maps, list(range(N_CORES)),
                               trace=do_trace)
    LAST_RESULTS = res

    out = np.zeros((T, D), dtype=np.float32)
    for e in range(E):
        cnt = dev_counts[e]
        if not cnt:
            continue
        ye = res.results[e]["y"][:cnt].astype(np.float32)
        if np.any(b2[e]):
            ye = ye + np.outer(wgt[e][:cnt], b2[e])
        out[idx[e][:cnt]] += ye

    # Host overflow path: exact fp32 FFN for tokens beyond expert capacity.
    for e in range(E):
        if counts[e] <= capT:
            continue
        oi = idx[e][capT:]
        ow = wgt[e][capT:]
        h = x_flat[oi] @ W1[e] + b1[e]
        h = 0.5 * h * (1.0 + _erf(h * np.float32(0.7071067811865476)))
        ye = h @ W2[e] + b2[e]
        out[oi] += ow[:, None] * ye

    return out.reshape(B, S, D)


# revision 48
# speedup vs baseline: 1.0804x; 1.0011x over previous
"""MoE layer (B=4,S=2048,D=1024,H=4096,E=8,K=2) on 8 trn2 NeuronCores.

Sharding strategy (hardcoded): expert-parallel with a fixed per-expert
capacity of 2048 tokens (= the perfectly load-balanced share: 8192 tokens
x top-2 / 8 experts). Host computes the gate (logits -> top-2 -> softmax
weights) and dispatches: core e receives up to 2048 tokens routed to
expert e (gathered + transposed + padded to the static capacity), plus
expert e's FFN weights in bf16. Each core runs the expert FFN
(x @ W1 -> gelu -> @ W2, fp32 PSUM accumulation) and scales rows by the
combine weight on-device. Tokens beyond an expert's capacity (the
capacity-factor-1.0 overflow, ~2% of pairs) take the host overflow path
(exact fp32 FFN). Host scatter-adds the weighted per-expert outputs back
into the full [B,S,D] output (the "all-to-all combine"), adding the b2
contribution exactly once per (token, expert) pair.

All device inputs are host-packed into SBUF layout ([128 partitions,
contiguous free dim]) so every DMA is 128 descriptors of >=1KB contiguous
runs: descriptor-generation (DIRECT2D) drops from ~5.5us to ~0.7us per
transfer and the transfers run at line rate.
"""

import sys

for _p in ("/opt/trn_rl_repo", "/root/.axon_site"):
    if _p not in sys.path:
        sys.path.insert(0, _p)

import numpy as np
import ml_dtypes

import concourse.bacc as bacc
import concourse.mybir as mybir
import concourse.tile as tile
from concourse.bass_utils import run_bass_kernel_spmd

BF16 = mybir.dt.bfloat16
F32 = mybir.dt.float32

N_CORES = 8
D = 1024
H = 4096
E = 8

_CACHE: dict = {}
LAST_RESULTS = None  # BassKernelResults of the most recent run (for test.py)
TRACE = False  # test.py can flip this to get an NTFF profile

try:
    from scipy.special import erf as _erf
except ImportError:
    import math

    def _erf(a):
        return np.vectorize(math.erf, otypes=[np.float32])(a)


def _build(capT, with_b1):
    nc = bacc.Bacc("TRN2", target_bir_lowering=False, debug=False,
                   num_devices=N_CORES)

    nblk = capT // 512
    # Packed layouts (built host-side):
    #   xp[p, blk, k, c]    = x_flat[token blk*512+c, d=k*128+p]
    #   w1p[p, g, q, k, c]  = W1[d=k*128+p, h=g*512+q*128+c]
    #   w2p[p, g, j, c]     = W2[h=(g*8+j)*128+p, d=c]
    # w1 is quarter(128-col)-granular so the first-matmul gate is only
    # x block0 (1MB) + one quarter (256KB); whole-group DMAs stay one
    # contiguous 8KB-run-per-partition transfer.
    xp_d = nc.dram_tensor("xp", [128, nblk, 8, 512], BF16, kind="ExternalInput")
    w1p_d = nc.dram_tensor("w1p", [128, 8, 4, 8, 128], BF16,
                           kind="ExternalInput")
    w2p_d = nc.dram_tensor("w2p", [128, 4, 8, 1024], BF16, kind="ExternalInput")
    wv_d = nc.dram_tensor("wv", [128, capT // 128], F32, kind="ExternalInput")
    if with_b1:
        b1_d = nc.dram_tensor("b1t", [128, 32], F32, kind="ExternalInput")
    # y ships back as bf16: halves the store traffic and the final store's
    # tail latency; the host combine accumulates in fp32 anyway.
    y_d = nc.dram_tensor("y", [capT, D], BF16, kind="ExternalOutput")

    with tile.TileContext(nc) as tc:
        with (
            tc.tile_pool(name="weights", bufs=1) as wpool,
            tc.tile_pool(name="xin", bufs=1) as xpool,
            tc.tile_pool(name="hbuf", bufs=2) as hpool,
            tc.tile_pool(name="yout", bufs=3) as ypool,
            tc.tile_pool(name="small", bufs=1) as spool,
            tc.tile_pool(name="ps1", bufs=3, space="PSUM") as ps1pool,
            tc.tile_pool(name="ps2", bufs=2, space="PSUM") as ps2pool,
        ):
            # Gating for the first L1 group is ~2MB (x block0 + w1 group0);
            # split it across the two HWDGE queues so each carries ~1MB and
            # the first matmuls can trickle on partial (k-sliced) data.
            xsb = {}
            xsb[0] = xpool.tile([128, 8, 512], BF16, tag="xT", name="xT0")
            w1g = []
            for g in range(8):
                t = wpool.tile([128, 4, 8, 128], BF16, tag=f"w1g{g}",
                               name=f"w1g{g}")
                w1g.append(t)
            nc.sync.dma_start(w1g[0][:, 0], w1p_d[:, 0, 0])
            nc.scalar.dma_start(xsb[0][:, :4, :], xp_d[:, 0, :4])
            nc.sync.dma_start(xsb[0][:, 4:, :], xp_d[:, 0, 4:])
            nc.scalar.dma_start(w1g[0][:, 1], w1p_d[:, 0, 1])
            nc.sync.dma_start(w1g[0][:, 2:], w1p_d[:, 0, 2:])
            if with_b1:
                b1_sb = spool.tile([128, 32], F32)
                nc.scalar.dma_start(b1_sb[:], b1_d[:])
            for g in range(1, 8):
                nc.sync.dma_start(w1g[g][:], w1p_d[:, g])
            w2g = []
            for g in range(4):
                t = wpool.tile([128, 8, 1024], BF16, tag=f"w2g{g}", name=f"w2g{g}")
                nc.sync.dma_start(t[:], w2p_d[:, g])
                w2g.append(t)

            wv_sb = spool.tile([128, capT // 128], F32)
            nc.sync.dma_start(wv_sb[:], wv_d[:])

            # HAM warm-up: the PE clock sits at 1.2GHz until it has been
            # busy ~3.4us. Spend that window on dummy matmuls while the
            # gating DMAs are in flight; sized so the burst ends right as
            # the first real tiles land (~12us), handing off without an
            # idle gap (an idle gap >3.4us would re-throttle the clock).
            warm_src = spool.tile([128, 128], BF16, name="warm_src")
            nc.gpsimd.memset(warm_src[:], 0.0)
            warm_ps = ps1pool.tile([128, 2, 512], F32, tag="ps1",
                                   name="warm_ps", bufs=None)
            for wi in range(36):
                nc.tensor.matmul(
                    warm_ps[:64, 0, :128], warm_src[:, :64], warm_src[:],
                    start=True, stop=True, skip_group_check=True)

            for blk in range(nblk):
                t0, tn = blk * 512, 512
                if blk not in xsb:
                    xsb[blk] = xpool.tile([128, 8, 512], BF16, tag="xT",
                                          name=f"xT{blk}")
                    nc.sync.dma_start(xsb[blk][:], xp_d[:, blk])
                xt = xsb[blk]

                # ---- layer 1: hT[m*128:(m+1)*128, :tn] for 32 H-tiles ----
                hT = hpool.tile([128, 32, 512], BF16, tag="hT", name=f"hT{blk}")
                for mg in range(16):
                    ps1 = ps1pool.tile([128, 2, 512], F32, tag="ps1",
                                       name=f"ps1_{blk}_{mg}")
                    for mj in range(2):
                        m = mg * 2 + mj
                        lg, q = m // 4, m % 4
                        for k in range(8):
                            nc.tensor.matmul(
                                ps1[:, mj, :tn],
                                w1g[lg][:, q, k, :],
                                xt[:, k, :tn],
                                start=(k == 0), stop=(k == 7),
                            )
                    if with_b1:
                        for mj in range(2):
                            m = mg * 2 + mj
                            nc.scalar.activation(
                                hT[:, m, :tn], ps1[:, mj, :tn],
                                mybir.ActivationFunctionType.Gelu,
                                bias=b1_sb[:, m:m + 1],
                            )
                    else:
                        nc.scalar.activation(
                            hT[:, mg * 2:mg * 2 + 2, :tn], ps1[:, :, :tn],
                            mybir.ActivationFunctionType.Gelu,
                        )

                # ---- layer 2: y[t0+tm*128 ..., :] = hT.T @ W2, scaled ----
                for tm in range(tn // 128):
                    col = t0 // 128 + tm
                    rows = slice(t0 + tm * 128, t0 + (tm + 1) * 128)
                    for dn in range(2):
                        ps2 = ps2pool.tile([128, 512], F32, tag="ps2",
                                           name=f"ps2_{blk}_{tm}_{dn}")
                        for h in range(32):
                            nc.tensor.matmul(
                                ps2[:, :],
                                hT[:, h, tm * 128:(tm + 1) * 128],
                                w2g[h // 8][:, h % 8, dn * 512:(dn + 1) * 512],
                                start=(h == 0), stop=(h == 31),
                            )
                        yt = ypool.tile([128, 512], BF16, tag="yt",
                                        name=f"yt_{blk}_{tm}_{dn}")
                        if blk == nblk - 1 and tm == tn // 128 - 1 and dn == 1:
                            # Tail: scale on ACT (faster PSUM read) and
                            # store from the same engine's queue, avoiding
                            # the cross-engine handoff on the last chain.
                            nc.scalar.activation(
                                yt[:], ps2[:],
                                mybir.ActivationFunctionType.Copy,
                                scale=wv_sb[:, col:col + 1])
                            nc.scalar.dma_start(y_d[rows, 512:1024], yt[:])
                        else:
                            nc.vector.tensor_scalar_mul(
                                yt[:], ps2[:], wv_sb[:, col:col + 1])
                            nc.sync.dma_start(
                                y_d[rows, dn * 512:(dn + 1) * 512], yt[:])

    nc.compile()
    return nc


def _route(x_flat, Wg, bg):
    """Host gate: returns per-expert (token_idx, combine_weight)."""
    logits = x_flat @ Wg.astype(np.float32) + bg.astype(np.float32)
    T = logits.shape[0]
    ar = np.arange(T)
    top1 = np.argmax(logits, axis=1)
    l2 = logits.copy()
    l2[ar, top1] = -np.inf
    top2 = np.argmax(l2, axis=1)
    v1 = logits[ar, top1]
    v2 = logits[ar, top2]
    # softmax over the two selected logits (v1 >= v2)
    e2 = np.exp(v2 - v1)
    s = 1.0 + e2
    wt1 = (1.0 / s).astype(np.float32)
    wt2 = (e2 / s).astype(np.float32)
    idx, wgt = [], []
    for e in range(E):
        m1 = top1 == e
        m2 = top2 == e
        ii = np.concatenate([ar[m1], ar[m2]])
        ww = np.concatenate([wt1[m1], wt2[m2]])
        order = np.argsort(ii, kind="stable")
        idx.append(ii[order])
        wgt.append(ww[order])
    return idx, wgt


def kernel(x, Wg, bg, W1, b1, W2, b2, _trace=None):
    global LAST_RESULTS
    x = np.asarray(x, dtype=np.float32)
    Wg = np.asarray(Wg, dtype=np.float32)
    bg = np.asarray(bg, dtype=np.float32)
    W1 = np.asarray(W1, dtype=np.float32)
    b1 = np.asarray(b1, dtype=np.float32)
    W2 = np.asarray(W2, dtype=np.float32)
    b2 = np.asarray(b2, dtype=np.float32)

    B, S, _D = x.shape
    T = B * S
    x_flat = np.ascontiguousarray(x.reshape(T, _D))

    idx, wgt = _route(x_flat, Wg, bg)
    counts = [len(i) for i in idx]
    # Fixed capacity = the load-balanced share (capacity factor 1.0).
    # Every core computes capT tokens regardless, so padding all cores to
    # the max expert count just burns PE time on zeros; overflow beyond
    # capT takes the exact host path instead.
    capT = min(2048, max(512, -(-max(counts) // 512) * 512))
    dev_counts = [min(c, capT) for c in counts]
    nblk = capT // 512

    with_b1 = bool(np.any(b1))
    key = (capT, with_b1)
    if key not in _CACHE:
        _CACHE[key] = _build(capT, with_b1)
    nc = _CACHE[key]

    bf = ml_dtypes.bfloat16
    in_maps = []
    for e in range(E):
        cnt = dev_counts[e]
        xT = np.zeros((D, capT), dtype=bf)
        if cnt:
            xT[:, :cnt] = x_flat[idx[e][:cnt]].T
        xp = xT.reshape(8, 128, nblk, 512).transpose(1, 2, 0, 3)
        w1p = W1[e].astype(bf).reshape(8, 128, 8, 4, 128).transpose(
            1, 2, 3, 0, 4)
        w2p = W2[e].astype(bf).reshape(4, 8, 128, 1024).transpose(2, 0, 1, 3)
        wv = np.zeros((capT // 128, 128), dtype=np.float32)
        if cnt:
            wv.reshape(-1)[:cnt] = wgt[e][:cnt]
        m = {
            "xp": np.ascontiguousarray(xp),
            "w1p": np.ascontiguousarray(w1p),
            "w2p": np.ascontiguousarray(w2p),
            "wv": np.ascontiguousarray(wv.T),
        }
        if with_b1:
            m["b1t"] = np.ascontiguousarray(b1[e].reshape(32, 128).T)
        in_maps.append(m)

    do_trace = TRACE if _trace is None else _trace
    res = run_bass_kernel_spmd(nc, in_maps, list(range(N_CORES)),
                               trace=do_trace)
    LAST_RESULTS = res

    out = np.zeros((T, D), dtype=np.float32)
    for e in range(E):
        cnt = dev_counts[e]
        if not cnt:
            continue
        ye = res.results[e]["y"][:cnt].astype(np.float32)
        if np.any(b2[e]):
            ye = ye + np.outer(wgt[e][:cnt], b2[e])
        out[idx[e][:cnt]] += ye

    # Host overflow path: exact fp32 FFN for tokens beyond expert capacity.
    for e in range(E):
        if counts[e] <= capT:
            continue
        oi = idx[e][capT:]
        ow = wgt[e][capT:]
        h = x_flat[oi] @ W1[e] + b1[e]
        h = 0.5 * h * (1.0 + _erf(h * np.float32(0.7071067811865476)))
        ye = h @ W2[e] + b2[e]
        out[oi] += ow[:, None] * ye

    return out.reshape(B, S, D)
